# revision 11
# baseline (speedup 1.0000x reference)
"""Multi-head causal self-attention with RoPE on 8 Trainium2 NeuronCores.

Problem: x[2,2048,2048], wq/wk/wv/wo[2048,2048] fp32, 16 heads (hd=128),
interleaved RoPE, causal softmax.

Sharding: 2-D (batch x head-group). Core = (b, hg): b = cix//4, hg = cix%4.
Each core handles ONE batch and FOUR heads (512 channels): wq/wk/wv
column-sharded, wo row-sharded; partial y summed on host per batch.
Halves x/y DMA vs pure head sharding and lets the V projection run at N=512.

All matmuls in bf16 (1 cycle/row at every moving size, FWL weight loads).
Tolerance is 2e-2; bf16 end-to-end lands ~4e-3.

Per-core layout:
  - xT = x[b]^T [d, s] bf16; wqkvT [d, 1536] = [q_h0..q_h3 (pre-scaled by
    1/sqrt(hd)), k_h0..k_h3, v 512ch]; woT [512, d]
  - projections: qT,kT per head-slice e via lhsT=w-tile, rhs=xT chunk
    -> [128, S] transposed; RoPE fused per chunk (signed pair-swap as a
    bf16 matmul + DVE cos/sin combine); v natural [s, 512] via
    lhsT=xT-subtile, rhs=wv (N=512)
  - attention per (j-block of 512 q), heads inner:
      scoresT[kv=128, q] = kT-tile.T @ qT-block, staircase causal tiles
      exp on ACT -> bf16 attn; triangle mask on Pool (gpsimd)
      rowsum WITHOUT matmul: DVE accumulates attn tiles into attn_sum,
      one ones-matmul per (j,h) turns it into per-q sums
      oT[d,q] += v-tile.T @ attn; normalize via DVE reciprocal+mult
      scores staggered 2 tiles ahead of attn@V so ACT exp never stalls PE
  - output projection for block j-1 emitted after attention block j
    (software pipeline hides the normalize latency); PSUM->SBUF y copies
    alternate ACT/DVE; y written bf16, host sums 4 partials per batch
"""

import os
import sys

for _p in ("/opt/trn_rl_repo", "/root/.axon_site/_ro/trn_rl_repo"):
    if os.path.isdir(_p) and _p not in sys.path:
        sys.path.append(_p)

import numpy as np

import concourse.bacc as bacc
import concourse.mybir as mybir
import concourse.tile as tile
from concourse.alu_op_type import AluOpType
from concourse.bass_utils import run_bass_kernel_spmd

F32 = mybir.dt.float32
BF16 = mybir.dt.bfloat16

B, S, D = 2, 2048, 2048
H, HD = 16, 128
NCORES = 8
HPC = 4                      # heads per core
CPC = HPC * HD               # 512 channels per core
P = 128
SC = 512                     # s-chunk / q-block
NSC = S // SC                # 4
NDT = D // P                 # 16 contraction tiles
NG = 2                       # x-tile DMA group
NQK = 2 * HPC                # 8 q/k e-slices of 128
WCOLS = NQK * P + CPC        # 1536
ROPE_THETA = 10000.0

Exp = mybir.ActivationFunctionType.Exp

last_exec_time_ns = None
_nc_cache = None


def _build_nc():
    nc = bacc.Bacc("TRN2", target_bir_lowering=False, debug=False)

    xT = nc.dram_tensor("xT", [D, S], BF16, kind="ExternalInput")
    wqkvT = nc.dram_tensor("wqkvT", [D, WCOLS], BF16, kind="ExternalInput")
    woT = nc.dram_tensor("woT", [CPC, D], BF16, kind="ExternalInput")
    cosT = nc.dram_tensor("cosT", [P, S], F32, kind="ExternalInput")
    sinT = nc.dram_tensor("sinT", [P, S], F32, kind="ExternalInput")
    rotL = nc.dram_tensor("rotL", [P, P], BF16, kind="ExternalInput")
    trimask = nc.dram_tensor("trimask", [P, P], BF16, kind="ExternalInput")
    ones = nc.dram_tensor("ones", [P, P], BF16, kind="ExternalInput")
    yT = nc.dram_tensor("yT", [D, S], BF16, kind="ExternalOutput")

    xTr = xT.rearrange("(o p) s -> p o s", p=P)

    with tile.TileContext(nc) as tc:
        with tc.tile_pool(name="const", bufs=1) as constp, \
             tc.tile_pool(name="xp", bufs=9) as xp, \
             tc.tile_pool(name="qk", bufs=1) as qkp, \
             tc.tile_pool(name="vp", bufs=1) as vp, \
             tc.tile_pool(name="op", bufs=1) as op_, \
             tc.tile_pool(name="attn", bufs=4) as attnp, \
             tc.tile_pool(name="sum", bufs=2) as sump, \
             tc.tile_pool(name="tmp", bufs=2) as tmpp, \
             tc.tile_pool(name="rec", bufs=2) as recp, \
             tc.tile_pool(name="yt", bufs=4) as ytp, \
             tc.tile_pool(name="ps", bufs=3, space="PSUM") as psp, \
             tc.tile_pool(name="acc", bufs=2, space="PSUM") as pacc, \
             tc.tile_pool(name="wo", bufs=2, space="PSUM") as pwo, \
             tc.tile_pool(name="rs", bufs=1, space="PSUM") as prsp:

            # ---- constants (wq split per d-tile so matmuls start early) ----
            wq_sb = constp.tile([P, NDT, WCOLS], BF16)
            wqr = wqkvT.rearrange("(o p) e -> p o e", p=P)
            for dt in range(NDT):
                nc.sync.dma_start(wq_sb[:, dt, :], wqr[:, dt, :])
            wo_sb = constp.tile([P, HPC, D], BF16)
            cos_sb = constp.tile([P, S], F32)
            sin_sb = constp.tile([P, S], F32)
            rot_sb = constp.tile([P, P], BF16)
            mask_sb = constp.tile([P, P], BF16)
            ones_sb = constp.tile([P, P], BF16)

            def load_rest_of_consts():
                nc.sync.dma_start(rot_sb[:], rotL[:])
                nc.sync.dma_start(cos_sb[:], cosT[:])
                nc.sync.dma_start(sin_sb[:], sinT[:])
                nc.sync.dma_start(mask_sb[:], trimask[:])
                nc.sync.dma_start(ones_sb[:], ones[:])
                nc.sync.dma_start(wo_sb[:], woT.rearrange("(o p) e -> p o e", p=P))

            # ---- projections (+ fused RoPE) ----
            qkT = [qkp.tile([P, S], BF16, tag=f"qk{e}", name=f"qkT{e}")
                   for e in range(NQK)]
            v_sb = vp.tile([P, NDT, CPC], BF16, tag="v")
            for sc in range(NSC):
                sl = slice(sc * SC, (sc + 1) * SC)
                xts = []
                for g in range(NDT // NG):
                    xt = xp.tile([P, NG, SC], BF16, tag="xt")
                    nc.gpsimd.dma_start(
                        xt[:], xTr[:, g * NG:(g + 1) * NG, sl])
                    xts.append(xt)
                if sc == 0:
                    load_rest_of_consts()
                for e in range(NQK):
                    pq = psp.tile([P, SC], F32, tag="ps")
                    for dt in range(NDT):
                        nc.tensor.matmul(pq[:],
                                         wq_sb[:, dt, e * P:(e + 1) * P],
                                         xts[dt // NG][:, dt % NG, :],
                                         start=(dt == 0), stop=(dt == NDT - 1))
                    nc.scalar.copy(qkT[e][:, sl], pq[:])
                    # RoPE for this chunk, overlapped with projections
                    pr = psp.tile([P, SC], F32, tag="ps")
                    nc.tensor.matmul(pr[:], rot_sb[:], qkT[e][:, sl],
                                     start=True, stop=True)
                    tmp = tmpp.tile([P, SC], F32, tag="ropetmp")
                    nc.vector.tensor_tensor(tmp[:], pr[:], sin_sb[:, sl],
                                            AluOpType.mult)
                    nc.vector.tensor_tensor(qkT[e][:, sl], qkT[e][:, sl],
                                            cos_sb[:, sl], AluOpType.mult)
                    nc.vector.tensor_tensor(qkT[e][:, sl], qkT[e][:, sl],
                                            tmp[:], AluOpType.add)
                for ss in range(SC // P):
                    pv = pacc.tile([P, SC], F32, tag="po")
                    for dt in range(NDT):
                        nc.tensor.matmul(pv[:],
                                         xts[dt // NG][:, dt % NG,
                                                       ss * P:(ss + 1) * P],
                                         wq_sb[:, dt, NQK * P:],
                                         start=(dt == 0), stop=(dt == NDT - 1))
                    nc.scalar.copy(v_sb[:, sc * (SC // P) + ss, :], pv[:])

            # ---- output projection for q-block j (called at j+1) ----
            oT = op_.tile([P, HPC, S], BF16, tag="o")

            def emit_wo(j, ehs):
                jsl = slice(j * SC, (j + 1) * SC)
                for eh in ehs:
                    yt = ytp.tile([P, 2, SC], BF16, tag="yt")
                    for si in range(2):
                        et = eh * 2 + si
                        py = pwo.tile([P, SC], F32, tag="pwo")
                        for ct in range(HPC):
                            nc.tensor.matmul(
                                py[:],
                                wo_sb[:, ct, et * P:(et + 1) * P],
                                oT[:, ct, jsl],
                                start=(ct == 0), stop=(ct == HPC - 1))
                        if si == 0:
                            nc.scalar.copy(yt[:, si, :], py[:])
                        else:
                            nc.vector.tensor_copy(yt[:, si, :], py[:])
                        # one 128KB DMA per et: the tail block stripes all
                        # 16 queues, ~6us drain instead of ~23
                        nc.sync.dma_start(
                            yT[et * P:(et + 1) * P, jsl], yt[:, si, :])

            # ---- attention: j outer, heads inner, scores 2 tiles ahead,
            #      softmax finalize deferred so PE never waits on it ----
            pending = [None]

            def finalize():
                if pending[0] is None:
                    return
                po, asum, osl = pending[0]
                pending[0] = None
                prs = prsp.tile([P, SC], F32, tag="prs")
                nc.tensor.matmul(prs[:], ones_sb[:], asum[:],
                                 start=True, stop=True)
                rec = recp.tile([P, SC], F32, tag="rec")
                nc.vector.reciprocal_approx_fast(rec[:], prs[:])
                nc.vector.tensor_tensor(osl, po[:], rec[:], AluOpType.mult)

            for j in range(NSC):
                jsl = slice(j * SC, (j + 1) * SC)
                n_kv = (SC // P) * (j + 1)
                for h in range(HPC):
                    qTh, kTh = qkT[h], qkT[HPC + h]
                    po = pacc.tile([P, SC], F32, tag="po")
                    asum = sump.tile([P, SC], BF16, tag="asum")
                    pscores = {}

                    def dlt_of(t):
                        return max(t - (SC // P) * j, 0) * P

                    def emit_score(t):
                        dlt = dlt_of(t)
                        ps_ = psp.tile([P, SC], F32, tag="ps")
                        nc.tensor.matmul(ps_[:, dlt:],
                                         kTh[:, t * P:(t + 1) * P],
                                         qTh[:, j * SC + dlt:(j + 1) * SC],
                                         start=True, stop=True)
                        pscores[t] = ps_

                    emit_score(0)
                    if n_kv > 1:
                        emit_score(1)
                    finalize()  # previous (j,h) — now covered by our scores
                    for t in range(n_kv):
                        if t + 2 < n_kv:
                            emit_score(t + 2)
                        dlt = dlt_of(t)
                        ps_ = pscores.pop(t)
                        attn = attnp.tile([P, SC], BF16, tag="attn")
                        nc.scalar.activation(attn[:, dlt:], ps_[:, dlt:],
                                             Exp, bias=0.0, scale=1.0)
                        if t - (SC // P) * j >= 0:
                            # triangle mask on the 128-col band (Pool engine)
                            nc.gpsimd.tensor_tensor(
                                attn[:, dlt:dlt + P], attn[:, dlt:dlt + P],
                                mask_sb[:], AluOpType.mult)
                        if t == 0:
                            nc.vector.tensor_copy(asum[:], attn[:])
                        else:
                            nc.vector.tensor_tensor(asum[:, dlt:],
                                                    asum[:, dlt:],
                                                    attn[:, dlt:],
                                                    AluOpType.add)
                        nc.tensor.matmul(po[:, dlt:],
                                         v_sb[:, t, h * HD:(h + 1) * HD],
                                         attn[:, dlt:],
                                         start=(t == 0), stop=(t == n_kv - 1),
                                         skip_group_check=True)
                    pending[0] = (po, asum, oT[:, h, jsl])
                    if j == NSC - 1 and h == HPC - 1:
                        # finalize under the last wo(j-1) chunk's matmuls
                        finalize()
                    # spread wo(j-1) between heads so its y DMAs drain
                    # during attention, not at the kernel tail
                    if j > 0:
                        emit_wo(j - 1, [2 * h, 2 * h + 1])
            finalize()
            emit_wo(NSC - 1, list(range(NDT // 2)))
    nc.finalize()
    return nc


def _host_inputs(x, wq, wk, wv, wo):
    """Build per-core input maps (host-side shard + transform)."""
    import ml_dtypes
    bf16 = ml_dtypes.bfloat16
    scale = 1.0 / np.sqrt(np.float32(HD))

    # RoPE tables, interleaved-pair layout [e, s]
    inv_freq = 1.0 / (ROPE_THETA ** (np.arange(0, HD, 2, dtype=np.float64) / HD))
    ang = np.arange(S, dtype=np.float64)[None, :] * inv_freq[:, None]  # [64, S]
    cosT = np.repeat(np.cos(ang), 2, axis=0).astype(np.float32)  # [128, S]
    sinT = np.repeat(np.sin(ang), 2, axis=0).astype(np.float32)

    # signed pair-swap: qrot[2i] = -q[2i+1], qrot[2i+1] = q[2i]
    rotL = np.zeros((HD, HD), dtype=np.float32)
    for i in range(HD // 2):
        rotL[2 * i + 1, 2 * i] = -1.0
        rotL[2 * i, 2 * i + 1] = 1.0
    rotL = rotL.astype(bf16)

    r = np.arange(P)[:, None]
    c = np.arange(P)[None, :]
    trimask = (c >= r).astype(bf16)
    ones_b = np.ones((P, P), dtype=np.float32).astype(bf16)

    wq_s = (wq * scale).astype(bf16)
    wk_s = wk.astype(bf16)
    wv_s = wv.astype(bf16)
    wo_s = wo.astype(bf16)

    xTb = [np.ascontiguousarray(x[b].T).astype(bf16) for b in range(B)]

    in_maps = []
    for cix in range(NCORES):
        b, hg = divmod(cix, NCORES // B)
        rows = slice(hg * CPC, (hg + 1) * CPC)
        blocks = []
        for h in range(HPC):
            hr = slice((hg * HPC + h) * HD, (hg * HPC + h + 1) * HD)
            blocks.append(wq_s[hr])
        for h in range(HPC):
            hr = slice((hg * HPC + h) * HD, (hg * HPC + h + 1) * HD)
            blocks.append(wk_s[hr])
        blocks.append(wv_s[rows])
        wqkvT = np.ascontiguousarray(
            np.concatenate(blocks, axis=0).T)  # [D, 1536]
        woT = np.ascontiguousarray(wo_s[:, rows].T)  # [512, D]
        in_maps.append({
            "xT": xTb[b],
            "wqkvT": wqkvT,
            "woT": woT,
            "cosT": cosT,
            "sinT": sinT,
            "rotL": rotL,
            "trimask": trimask,
            "ones": ones_b,
        })
    return in_maps


def _get_nc():
    global _nc_cache
    if _nc_cache is None:
        _nc_cache = _build_nc()
    return _nc_cache


def kernel(x, wq, wk, wv, wo, _trace=False):
    global last_exec_time_ns
    nc = _get_nc()
    in_maps = _host_inputs(np.asarray(x, dtype=np.float32),
                           np.asarray(wq, dtype=np.float32),
                           np.asarray(wk, dtype=np.float32),
                           np.asarray(wv, dtype=np.float32),
                           np.asarray(wo, dtype=np.float32))
    res = run_bass_kernel_spmd(nc, in_maps, core_ids=list(range(NCORES)),
                               trace=_trace)
    last_exec_time_ns = res.exec_time_ns
    y = np.zeros((B, S, D), dtype=np.float32)
    for cix in range(NCORES):
        b = cix // (NCORES // B)
        y[b] += res.results[cix]["yT"].T.astype(np.float32)
    return y


# revision 14
# speedup vs baseline: 1.1751x; 1.1751x over previous
"""Multi-head causal self-attention with RoPE on 8 Trainium2 NeuronCores.

Problem: x[2,2048,2048], wq/wk/wv/wo[2048,2048] fp32, 16 heads (hd=128),
interleaved RoPE, causal softmax.

Sharding: 2-D (batch x head-group). Core = (b, hg): b = cix//4, hg = cix%4.
Each core handles ONE batch and FOUR heads (512 channels): wq/wk/wv
column-sharded, wo row-sharded; partial y summed on host per batch.
Halves x/y DMA vs pure head sharding and lets the V projection run at N=512.

All matmuls in bf16 (1 cycle/row at every moving size, FWL weight loads).
Tolerance is 2e-2; bf16 end-to-end lands ~4e-3.

Per-core layout:
  - xT = x[b]^T [d, s] bf16; wqkvT [d, 1536] = [q_h0..q_h3 (pre-scaled by
    1/sqrt(hd)), k_h0..k_h3, v 512ch]; woT [512, d]
  - projections: qT,kT per head-slice e via lhsT=w-tile, rhs=xT chunk
    -> [128, S] transposed; RoPE fused per chunk (signed pair-swap as a
    bf16 matmul + DVE cos/sin combine); v natural [s, 512] via
    lhsT=xT-subtile, rhs=wv (N=512)
  - attention per (j-block of 512 q), heads inner:
      scoresT[kv=128, q] = kT-tile.T @ qT-block, staircase causal tiles
      exp on ACT -> bf16 attn; triangle mask on Pool (gpsimd)
      rowsum WITHOUT matmul: DVE accumulates attn tiles into attn_sum,
      one ones-matmul per (j,h) turns it into per-q sums
      oT[d,q] += v-tile.T @ attn; normalize via DVE reciprocal+mult
      scores staggered 2 tiles ahead of attn@V so ACT exp never stalls PE
  - output projection for block j-1 emitted after attention block j
    (software pipeline hides the normalize latency); PSUM->SBUF y copies
    alternate ACT/DVE; y written bf16, host sums 4 partials per batch
"""

import os
import sys

for _p in ("/opt/trn_rl_repo", "/root/.axon_site/_ro/trn_rl_repo"):
    if os.path.isdir(_p) and _p not in sys.path:
        sys.path.append(_p)

import numpy as np

import concourse.bacc as bacc
import concourse.mybir as mybir
import concourse.tile as tile
from concourse.alu_op_type import AluOpType
from concourse.bass_utils import run_bass_kernel_spmd

F32 = mybir.dt.float32
BF16 = mybir.dt.bfloat16

B, S, D = 2, 2048, 2048
H, HD = 16, 128
NCORES = 8
HPC = 4                      # heads per core
CPC = HPC * HD               # 512 channels per core
P = 128
SC = 512                     # s-chunk / q-block
NSC = S // SC                # 4
NDT = D // P                 # 16 contraction tiles
NG = 2                       # x-tile DMA group
NQK = 2 * HPC                # 8 q/k e-slices of 128
WCOLS = NQK * P + CPC        # 1536
ROPE_THETA = 10000.0

Exp = mybir.ActivationFunctionType.Exp

last_exec_time_ns = None
_nc_cache = None


def _build_nc():
    nc = bacc.Bacc("TRN2", target_bir_lowering=False, debug=False)

    xT = nc.dram_tensor("xT", [D, S], BF16, kind="ExternalInput")
    wqkvT = nc.dram_tensor("wqkvT", [D, WCOLS], BF16, kind="ExternalInput")
    woT = nc.dram_tensor("woT", [CPC, D], BF16, kind="ExternalInput")
    cosT = nc.dram_tensor("cosT", [P, S], F32, kind="ExternalInput")
    sinT = nc.dram_tensor("sinT", [P, S], F32, kind="ExternalInput")
    rotL = nc.dram_tensor("rotL", [P, P], BF16, kind="ExternalInput")
    trimask = nc.dram_tensor("trimask", [P, P], BF16, kind="ExternalInput")
    ones = nc.dram_tensor("ones", [P, P], BF16, kind="ExternalInput")
    yT = nc.dram_tensor("yT", [D, S], BF16, kind="ExternalOutput")

    xTr = xT.rearrange("(o p) s -> p o s", p=P)

    with tile.TileContext(nc) as tc:
        with tc.tile_pool(name="const", bufs=1) as constp, \
             tc.tile_pool(name="xp", bufs=9) as xp, \
             tc.tile_pool(name="qk", bufs=1) as qkp, \
             tc.tile_pool(name="vp", bufs=1) as vp, \
             tc.tile_pool(name="op", bufs=1) as op_, \
             tc.tile_pool(name="attn", bufs=4) as attnp, \
             tc.tile_pool(name="sum", bufs=2) as sump, \
             tc.tile_pool(name="tmp", bufs=2) as tmpp, \
             tc.tile_pool(name="rec", bufs=2) as recp, \
             tc.tile_pool(name="yt", bufs=4) as ytp, \
             tc.tile_pool(name="ps", bufs=3, space="PSUM") as psp, \
             tc.tile_pool(name="acc", bufs=2, space="PSUM") as pacc, \
             tc.tile_pool(name="wo", bufs=2, space="PSUM") as pwo, \
             tc.tile_pool(name="rs", bufs=1, space="PSUM") as prsp:

            # ---- constants (wq split per d-tile so matmuls start early) ----
            wq_sb = constp.tile([P, NDT, WCOLS], BF16)
            wqr = wqkvT.rearrange("(o p) e -> p o e", p=P)
            for dt in range(NDT):
                nc.sync.dma_start(wq_sb[:, dt, :], wqr[:, dt, :])
            wo_sb = constp.tile([P, HPC, D], BF16)
            cos_sb = constp.tile([P, S], F32)
            sin_sb = constp.tile([P, S], F32)
            rot_sb = constp.tile([P, P], BF16)
            mask_sb = constp.tile([P, P], BF16)
            ones_sb = constp.tile([P, P], BF16)

            def load_rest_of_consts():
                nc.sync.dma_start(rot_sb[:], rotL[:])
                nc.sync.dma_start(cos_sb[:], cosT[:])
                nc.sync.dma_start(sin_sb[:], sinT[:])
                nc.sync.dma_start(mask_sb[:], trimask[:])
                nc.sync.dma_start(ones_sb[:], ones[:])
                nc.sync.dma_start(wo_sb[:], woT.rearrange("(o p) e -> p o e", p=P))

            # ---- projections (+ fused RoPE) ----
            qkT = [qkp.tile([P, S], BF16, tag=f"qk{e}", name=f"qkT{e}")
                   for e in range(NQK)]
            v_sb = vp.tile([P, NDT, CPC], BF16, tag="v")
            for sc in range(NSC):
                sl = slice(sc * SC, (sc + 1) * SC)
                xts = []
                for g in range(NDT // NG):
                    xt = xp.tile([P, NG, SC], BF16, tag="xt")
                    nc.gpsimd.dma_start(
                        xt[:], xTr[:, g * NG:(g + 1) * NG, sl])
                    xts.append(xt)
                if sc == 0:
                    load_rest_of_consts()
                for e in range(NQK):
                    pq = psp.tile([P, SC], F32, tag="ps")
                    for dt in range(NDT):
                        nc.tensor.matmul(pq[:],
                                         wq_sb[:, dt, e * P:(e + 1) * P],
                                         xts[dt // NG][:, dt % NG, :],
                                         start=(dt == 0), stop=(dt == NDT - 1))
                    nc.scalar.copy(qkT[e][:, sl], pq[:])
                    # RoPE for this chunk, overlapped with projections
                    pr = psp.tile([P, SC], F32, tag="ps")
                    nc.tensor.matmul(pr[:], rot_sb[:], qkT[e][:, sl],
                                     start=True, stop=True)
                    tmp = tmpp.tile([P, SC], F32, tag="ropetmp")
                    nc.vector.tensor_tensor(tmp[:], pr[:], sin_sb[:, sl],
                                            AluOpType.mult)
                    nc.vector.tensor_tensor(qkT[e][:, sl], qkT[e][:, sl],
                                            cos_sb[:, sl], AluOpType.mult)
                    nc.vector.tensor_tensor(qkT[e][:, sl], qkT[e][:, sl],
                                            tmp[:], AluOpType.add)
                for ss in range(SC // P):
                    pv = pacc.tile([P, SC], F32, tag="po")
                    for dt in range(NDT):
                        nc.tensor.matmul(pv[:],
                                         xts[dt // NG][:, dt % NG,
                                                       ss * P:(ss + 1) * P],
                                         wq_sb[:, dt, NQK * P:],
                                         start=(dt == 0), stop=(dt == NDT - 1))
                    nc.scalar.copy(v_sb[:, sc * (SC // P) + ss, :], pv[:])

            # ---- output projection for q-block j (called at j+1) ----
            # one oT tile per j-block: wo(j-1) must not falsely depend on
            # norm writes of block j (dep tracking is tile-granular)
            oTs = [op_.tile([P, HPC, SC], BF16, tag=f"o{j}", name=f"oT{j}")
                   for j in range(NSC)]

            def emit_wo(j, ehs):
                jsl = slice(j * SC, (j + 1) * SC)
                oT = oTs[j]
                for eh in ehs:
                    yt = ytp.tile([P, 2, SC], BF16, tag="yt")
                    for si in range(2):
                        et = eh * 2 + si
                        py = pwo.tile([P, SC], F32, tag="pwo")
                        for ct in range(HPC):
                            nc.tensor.matmul(
                                py[:],
                                wo_sb[:, ct, et * P:(et + 1) * P],
                                oT[:, ct, :],
                                start=(ct == 0), stop=(ct == HPC - 1))
                        if si == 0:
                            nc.scalar.copy(yt[:, si, :], py[:])
                        else:
                            nc.vector.tensor_copy(yt[:, si, :], py[:])
                        # one 128KB DMA per et: the tail block stripes all
                        # 16 queues, ~6us drain instead of ~23
                        nc.sync.dma_start(
                            yT[et * P:(et + 1) * P, jsl], yt[:, si, :])

            # ---- attention: j outer, heads inner, scores 2 tiles ahead,
            #      softmax finalize deferred so PE never waits on it ----
            pending = [None]

            def finalize():
                if pending[0] is None:
                    return
                po, asum, osl = pending[0]
                pending[0] = None
                prs = prsp.tile([P, SC], F32, tag="prs")
                nc.tensor.matmul(prs[:], ones_sb[:], asum[:],
                                 start=True, stop=True)
                rec = recp.tile([P, SC], F32, tag="rec")
                nc.vector.reciprocal_approx_fast(rec[:], prs[:])
                nc.vector.tensor_tensor(osl, po[:], rec[:], AluOpType.mult)

            for j in range(NSC):
                jsl = slice(j * SC, (j + 1) * SC)
                n_kv = (SC // P) * (j + 1)
                for h in range(HPC):
                    qTh, kTh = qkT[h], qkT[HPC + h]
                    po = pacc.tile([P, SC], F32, tag="po")
                    asum = sump.tile([P, SC], BF16, tag="asum")
                    pscores = {}

                    def dlt_of(t):
                        return max(t - (SC // P) * j, 0) * P

                    def emit_score(t):
                        dlt = dlt_of(t)
                        ps_ = psp.tile([P, SC], F32, tag="ps")
                        nc.tensor.matmul(ps_[:, dlt:],
                                         kTh[:, t * P:(t + 1) * P],
                                         qTh[:, j * SC + dlt:(j + 1) * SC],
                                         start=True, stop=True)
                        pscores[t] = ps_

                    emit_score(0)
                    if n_kv > 1:
                        emit_score(1)
                    finalize()  # previous (j,h) — now covered by our scores
                    for t in range(n_kv):
                        if t + 2 < n_kv:
                            emit_score(t + 2)
                        dlt = dlt_of(t)
                        ps_ = pscores.pop(t)
                        attn = attnp.tile([P, SC], BF16, tag="attn")
                        nc.scalar.activation(attn[:, dlt:], ps_[:, dlt:],
                                             Exp, bias=0.0, scale=1.0)
                        if t - (SC // P) * j >= 0:
                            # triangle mask on the 128-col band (Pool engine)
                            nc.gpsimd.tensor_tensor(
                                attn[:, dlt:dlt + P], attn[:, dlt:dlt + P],
                                mask_sb[:], AluOpType.mult)
                        if t == 0:
                            nc.vector.tensor_copy(asum[:], attn[:])
                        else:
                            nc.vector.tensor_tensor(asum[:, dlt:],
                                                    asum[:, dlt:],
                                                    attn[:, dlt:],
                                                    AluOpType.add)
                        nc.tensor.matmul(po[:, dlt:],
                                         v_sb[:, t, h * HD:(h + 1) * HD],
                                         attn[:, dlt:],
                                         start=(t == 0), stop=(t == n_kv - 1),
                                         skip_group_check=True)
                    pending[0] = (po, asum, oTs[j][:, h, :])
                    if j == NSC - 1 and h == HPC - 1:
                        # finalize under the last wo(j-1) chunk's matmuls
                        finalize()
                    # spread wo(j-1) between heads so its y DMAs drain
                    # during attention, not at the kernel tail
                    if j > 0:
                        emit_wo(j - 1, [2 * h, 2 * h + 1])
            finalize()
            emit_wo(NSC - 1, list(range(NDT // 2)))
    nc.finalize()
    return nc


def _host_inputs(x, wq, wk, wv, wo):
    """Build per-core input maps (host-side shard + transform)."""
    import ml_dtypes
    bf16 = ml_dtypes.bfloat16
    scale = 1.0 / np.sqrt(np.float32(HD))

    # RoPE tables, interleaved-pair layout [e, s]
    inv_freq = 1.0 / (ROPE_THETA ** (np.arange(0, HD, 2, dtype=np.float64) / HD))
    ang = np.arange(S, dtype=np.float64)[None, :] * inv_freq[:, None]  # [64, S]
    cosT = np.repeat(np.cos(ang), 2, axis=0).astype(np.float32)  # [128, S]
    sinT = np.repeat(np.sin(ang), 2, axis=0).astype(np.float32)

    # signed pair-swap: qrot[2i] = -q[2i+1], qrot[2i+1] = q[2i]
    rotL = np.zeros((HD, HD), dtype=np.float32)
    for i in range(HD // 2):
        rotL[2 * i + 1, 2 * i] = -1.0
        rotL[2 * i, 2 * i + 1] = 1.0
    rotL = rotL.astype(bf16)

    r = np.arange(P)[:, None]
    c = np.arange(P)[None, :]
    trimask = (c >= r).astype(bf16)
    ones_b = np.ones((P, P), dtype=np.float32).astype(bf16)

    wq_s = (wq * scale).astype(bf16)
    wk_s = wk.astype(bf16)
    wv_s = wv.astype(bf16)
    wo_s = wo.astype(bf16)

    xTb = [np.ascontiguousarray(x[b].T).astype(bf16) for b in range(B)]

    in_maps = []
    for cix in range(NCORES):
        b, hg = divmod(cix, NCORES // B)
        rows = slice(hg * CPC, (hg + 1) * CPC)
        blocks = []
        for h in range(HPC):
            hr = slice((hg * HPC + h) * HD, (hg * HPC + h + 1) * HD)
            blocks.append(wq_s[hr])
        for h in range(HPC):
            hr = slice((hg * HPC + h) * HD, (hg * HPC + h + 1) * HD)
            blocks.append(wk_s[hr])
        blocks.append(wv_s[rows])
        wqkvT = np.ascontiguousarray(
            np.concatenate(blocks, axis=0).T)  # [D, 1536]
        woT = np.ascontiguousarray(wo_s[:, rows].T)  # [512, D]
        in_maps.append({
            "xT": xTb[b],
            "wqkvT": wqkvT,
            "woT": woT,
            "cosT": cosT,
            "sinT": sinT,
            "rotL": rotL,
            "trimask": trimask,
            "ones": ones_b,
        })
    return in_maps


def _get_nc():
    global _nc_cache
    if _nc_cache is None:
        _nc_cache = _build_nc()
    return _nc_cache


def kernel(x, wq, wk, wv, wo, _trace=False):
    global last_exec_time_ns
    nc = _get_nc()
    in_maps = _host_inputs(np.asarray(x, dtype=np.float32),
                           np.asarray(wq, dtype=np.float32),
                           np.asarray(wk, dtype=np.float32),
                           np.asarray(wv, dtype=np.float32),
                           np.asarray(wo, dtype=np.float32))
    res = run_bass_kernel_spmd(nc, in_maps, core_ids=list(range(NCORES)),
                               trace=_trace)
    last_exec_time_ns = res.exec_time_ns
    y = np.zeros((B, S, D), dtype=np.float32)
    for cix in range(NCORES):
        b = cix // (NCORES // B)
        y[b] += res.results[cix]["yT"].T.astype(np.float32)
    return y


# revision 18
# speedup vs baseline: 1.1931x; 1.0153x over previous
"""Multi-head causal self-attention with RoPE on 8 Trainium2 NeuronCores.

Problem: x[2,2048,2048], wq/wk/wv/wo[2048,2048] fp32, 16 heads (hd=128),
interleaved RoPE, causal softmax.

Sharding: 2-D (batch x head-group). Core = (b, hg): b = cix//4, hg = cix%4.
Each core handles ONE batch and FOUR heads (512 channels): wq/wk/wv
column-sharded, wo row-sharded; partial y summed on host per batch.
Halves x/y DMA vs pure head sharding and lets the V projection run at N=512.

All matmuls in bf16 (1 cycle/row at every moving size, FWL weight loads).
Tolerance is 2e-2; bf16 end-to-end lands ~4e-3.

Per-core layout:
  - xT = x[b]^T [d, s] bf16; wqkvT [d, 1536] = [q_h0..q_h3 (pre-scaled by
    1/sqrt(hd)), k_h0..k_h3, v 512ch]; woT [512, d]
  - projections: qT,kT per head-slice e via lhsT=w-tile, rhs=xT chunk
    -> [128, S] transposed; RoPE fused per chunk (signed pair-swap as a
    bf16 matmul + DVE cos/sin combine); v natural [s, 512] via
    lhsT=xT-subtile, rhs=wv (N=512)
  - attention per (j-block of 512 q), heads inner:
      scoresT[kv=128, q] = kT-tile.T @ qT-block, staircase causal tiles
      exp on ACT -> bf16 attn; triangle mask on Pool (gpsimd)
      rowsum WITHOUT matmul: DVE accumulates attn tiles into attn_sum,
      one ones-matmul per (j,h) turns it into per-q sums
      oT[d,q] += v-tile.T @ attn; normalize via DVE reciprocal+mult
      scores staggered 2 tiles ahead of attn@V so ACT exp never stalls PE
  - output projection for block j-1 emitted after attention block j
    (software pipeline hides the normalize latency); PSUM->SBUF y copies
    alternate ACT/DVE; y written bf16, host sums 4 partials per batch
"""

import os
import sys

for _p in ("/opt/trn_rl_repo", "/root/.axon_site/_ro/trn_rl_repo"):
    if os.path.isdir(_p) and _p not in sys.path:
        sys.path.append(_p)

import numpy as np

import concourse.bacc as bacc
import concourse.mybir as mybir
import concourse.tile as tile
from concourse.alu_op_type import AluOpType
from concourse.bass_utils import run_bass_kernel_spmd

F32 = mybir.dt.float32
BF16 = mybir.dt.bfloat16

B, S, D = 2, 2048, 2048
H, HD = 16, 128
NCORES = 8
HPC = 4                      # heads per core
CPC = HPC * HD               # 512 channels per core
P = 128
SC = 512                     # s-chunk / q-block
NSC = S // SC                # 4
NDT = D // P                 # 16 contraction tiles
NG = 2                       # x-tile DMA group
NQK = 2 * HPC                # 8 q/k e-slices of 128
WCOLS = NQK * P + CPC        # 1536
ROPE_THETA = 10000.0

Exp = mybir.ActivationFunctionType.Exp

last_exec_time_ns = None
_nc_cache = None


def _build_nc():
    nc = bacc.Bacc("TRN2", target_bir_lowering=False, debug=False)

    xT = nc.dram_tensor("xT", [D, S], BF16, kind="ExternalInput")
    wqkvT = nc.dram_tensor("wqkvT", [D, WCOLS], BF16, kind="ExternalInput")
    woT = nc.dram_tensor("woT", [CPC, D], BF16, kind="ExternalInput")
    cosT = nc.dram_tensor("cosT", [P, S], F32, kind="ExternalInput")
    sinT = nc.dram_tensor("sinT", [P, S], F32, kind="ExternalInput")
    rotL = nc.dram_tensor("rotL", [P, P], BF16, kind="ExternalInput")
    trimask = nc.dram_tensor("trimask", [P, P], BF16, kind="ExternalInput")
    ones = nc.dram_tensor("ones", [P, P], BF16, kind="ExternalInput")
    yT = nc.dram_tensor("yT", [D, S], BF16, kind="ExternalOutput")

    xTr = xT.rearrange("(o p) s -> p o s", p=P)

    with tile.TileContext(nc) as tc:
        with tc.tile_pool(name="const", bufs=1) as constp, \
             tc.tile_pool(name="xp", bufs=13) as xp, \
             tc.tile_pool(name="qk", bufs=1) as qkp, \
             tc.tile_pool(name="vp", bufs=1) as vp, \
             tc.tile_pool(name="op", bufs=1) as op_, \
             tc.tile_pool(name="attn", bufs=6) as attnp, \
             tc.tile_pool(name="sum", bufs=2) as sump, \
             tc.tile_pool(name="tmp", bufs=2) as tmpp, \
             tc.tile_pool(name="rec", bufs=2) as recp, \
             tc.tile_pool(name="yt", bufs=4) as ytp, \
             tc.tile_pool(name="ps", bufs=3, space="PSUM") as psp, \
             tc.tile_pool(name="acc", bufs=2, space="PSUM") as pacc, \
             tc.tile_pool(name="wo", bufs=2, space="PSUM") as pwo, \
             tc.tile_pool(name="rs", bufs=1, space="PSUM") as prsp:

            # ---- constants (wq per d-tile, interleaved with the first x
            #      chunk's DMAs below so the first accumulation streams) ----
            wq_sb = constp.tile([P, NDT, WCOLS], BF16)
            wqr = wqkvT.rearrange("(o p) e -> p o e", p=P)
            wo_sb = constp.tile([P, HPC, D], BF16)
            cos_sb = constp.tile([P, S], F32)
            sin_sb = constp.tile([P, S], F32)
            rot_sb = constp.tile([P, P], BF16)
            mask_sb = constp.tile([P, P], BF16)
            ones_sb = constp.tile([P, P], BF16)

            def load_rest_of_consts():
                nc.sync.dma_start(rot_sb[:], rotL[:])
                nc.sync.dma_start(cos_sb[:], cosT[:])
                nc.sync.dma_start(sin_sb[:], sinT[:])
                nc.sync.dma_start(mask_sb[:], trimask[:])
                nc.sync.dma_start(ones_sb[:], ones[:])
                nc.sync.dma_start(wo_sb[:], woT.rearrange("(o p) e -> p o e", p=P))

            # ---- projections (+ fused RoPE) ----
            qkT = [qkp.tile([P, S], BF16, tag=f"qk{e}", name=f"qkT{e}")
                   for e in range(NQK)]
            v_sb = vp.tile([P, NDT, CPC], BF16, tag="v")
            for sc in range(NSC):
                sl = slice(sc * SC, (sc + 1) * SC)
                xts = []
                for g in range(NDT // NG):
                    if sc == 0:
                        for i in range(NG):
                            dt = g * NG + i
                            nc.sync.dma_start(wq_sb[:, dt, :], wqr[:, dt, :])
                    xt = xp.tile([P, NG, SC], BF16, tag="xt")
                    nc.gpsimd.dma_start(
                        xt[:], xTr[:, g * NG:(g + 1) * NG, sl])
                    xts.append(xt)
                if sc == 0:
                    load_rest_of_consts()
                for e in range(NQK):
                    pq = psp.tile([P, SC], F32, tag="ps")
                    for dt in range(NDT):
                        nc.tensor.matmul(pq[:],
                                         wq_sb[:, dt, e * P:(e + 1) * P],
                                         xts[dt // NG][:, dt % NG, :],
                                         start=(dt == 0), stop=(dt == NDT - 1))
                    nc.scalar.copy(qkT[e][:, sl], pq[:])
                    # RoPE for this chunk, overlapped with projections
                    pr = psp.tile([P, SC], F32, tag="ps")
                    nc.tensor.matmul(pr[:], rot_sb[:], qkT[e][:, sl],
                                     start=True, stop=True)
                    tmp = tmpp.tile([P, SC], F32, tag="ropetmp")
                    nc.vector.tensor_tensor(tmp[:], pr[:], sin_sb[:, sl],
                                            AluOpType.mult)
                    nc.vector.tensor_tensor(qkT[e][:, sl], qkT[e][:, sl],
                                            cos_sb[:, sl], AluOpType.mult)
                    nc.vector.tensor_tensor(qkT[e][:, sl], qkT[e][:, sl],
                                            tmp[:], AluOpType.add)
                for ss in range(SC // P):
                    pv = pacc.tile([P, SC], F32, tag="po")
                    for dt in range(NDT):
                        nc.tensor.matmul(pv[:],
                                         xts[dt // NG][:, dt % NG,
                                                       ss * P:(ss + 1) * P],
                                         wq_sb[:, dt, NQK * P:],
                                         start=(dt == 0), stop=(dt == NDT - 1))
                    nc.scalar.copy(v_sb[:, sc * (SC // P) + ss, :], pv[:])

            # ---- output projection for q-block j (called at j+1) ----
            # one oT tile per j-block: wo(j-1) must not falsely depend on
            # norm writes of block j (dep tracking is tile-granular)
            oTs = [op_.tile([P, HPC, SC], BF16, tag=f"o{j}", name=f"oT{j}")
                   for j in range(NSC)]

            def emit_wo(j, ehs):
                jsl = slice(j * SC, (j + 1) * SC)
                oT = oTs[j]
                for eh in ehs:
                    yt = ytp.tile([P, 2, SC], BF16, tag="yt")
                    for si in range(2):
                        et = eh * 2 + si
                        py = pwo.tile([P, SC], F32, tag="pwo")
                        for ct in range(HPC):
                            nc.tensor.matmul(
                                py[:],
                                wo_sb[:, ct, et * P:(et + 1) * P],
                                oT[:, ct, :],
                                start=(ct == 0), stop=(ct == HPC - 1))
                        if si == 0:
                            nc.scalar.copy(yt[:, si, :], py[:])
                        else:
                            nc.vector.tensor_copy(yt[:, si, :], py[:])
                        # one 128KB DMA per et: the tail block stripes all
                        # 16 queues, ~6us drain instead of ~23
                        nc.sync.dma_start(
                            yT[et * P:(et + 1) * P, jsl], yt[:, si, :])

            # ---- attention: j outer, heads inner, scores 2 tiles ahead,
            #      softmax finalize deferred so PE never waits on it ----
            pending = [None]

            def finalize():
                if pending[0] is None:
                    return
                po, asum, osl = pending[0]
                pending[0] = None
                prs = prsp.tile([P, SC], F32, tag="prs")
                nc.tensor.matmul(prs[:], ones_sb[:], asum[:],
                                 start=True, stop=True)
                rec = recp.tile([P, SC], F32, tag="rec")
                nc.vector.reciprocal_approx_fast(rec[:], prs[:])
                nc.vector.tensor_tensor(osl, po[:], rec[:], AluOpType.mult)

            for j in range(NSC):
                jsl = slice(j * SC, (j + 1) * SC)
                n_kv = (SC // P) * (j + 1)
                for h in range(HPC):
                    qTh, kTh = qkT[h], qkT[HPC + h]
                    po = pacc.tile([P, SC], F32, tag="po")
                    asum = sump.tile([P, SC], BF16, tag="asum")
                    pscores = {}

                    def dlt_of(t):
                        return max(t - (SC // P) * j, 0) * P

                    def emit_score(t):
                        dlt = dlt_of(t)
                        ps_ = psp.tile([P, SC], F32, tag="ps")
                        nc.tensor.matmul(ps_[:, dlt:],
                                         kTh[:, t * P:(t + 1) * P],
                                         qTh[:, j * SC + dlt:(j + 1) * SC],
                                         start=True, stop=True)
                        pscores[t] = ps_

                    emit_score(0)
                    if n_kv > 1:
                        emit_score(1)
                    finalize()  # previous (j,h) — now covered by our scores
                    for t in range(n_kv):
                        if t + 2 < n_kv:
                            emit_score(t + 2)
                        dlt = dlt_of(t)
                        ps_ = pscores.pop(t)
                        attn = attnp.tile([P, SC], BF16, tag="attn")
                        nc.scalar.activation(attn[:, dlt:], ps_[:, dlt:],
                                             Exp, bias=0.0, scale=1.0)
                        if t - (SC // P) * j >= 0:
                            # triangle mask on the 128-col band (Pool engine)
                            nc.gpsimd.tensor_tensor(
                                attn[:, dlt:dlt + P], attn[:, dlt:dlt + P],
                                mask_sb[:], AluOpType.mult)
                        if t == 0:
                            nc.vector.tensor_copy(asum[:], attn[:])
                        else:
                            nc.vector.tensor_tensor(asum[:, dlt:],
                                                    asum[:, dlt:],
                                                    attn[:, dlt:],
                                                    AluOpType.add)
                        nc.tensor.matmul(po[:, dlt:],
                                         v_sb[:, t, h * HD:(h + 1) * HD],
                                         attn[:, dlt:],
                                         start=(t == 0), stop=(t == n_kv - 1),
                                         skip_group_check=True)
                    pending[0] = (po, asum, oTs[j][:, h, :])
                    if j == NSC - 1 and h == HPC - 1:
                        # finalize under the last wo(j-1) chunk's matmuls
                        finalize()
                    # spread wo(j-1) between heads so its y DMAs drain
                    # during attention, not at the kernel tail
                    if j > 0:
                        emit_wo(j - 1, [2 * h, 2 * h + 1])
            finalize()
            emit_wo(NSC - 1, list(range(NDT // 2)))
    nc.finalize()
    return nc


def _host_inputs(x, wq, wk, wv, wo):
    """Build per-core input maps (host-side shard + transform)."""
    import ml_dtypes
    bf16 = ml_dtypes.bfloat16
    scale = 1.0 / np.sqrt(np.float32(HD))

    # RoPE tables, interleaved-pair layout [e, s]
    inv_freq = 1.0 / (ROPE_THETA ** (np.arange(0, HD, 2, dtype=np.float64) / HD))
    ang = np.arange(S, dtype=np.float64)[None, :] * inv_freq[:, None]  # [64, S]
    cosT = np.repeat(np.cos(ang), 2, axis=0).astype(np.float32)  # [128, S]
    sinT = np.repeat(np.sin(ang), 2, axis=0).astype(np.float32)

    # signed pair-swap: qrot[2i] = -q[2i+1], qrot[2i+1] = q[2i]
    rotL = np.zeros((HD, HD), dtype=np.float32)
    for i in range(HD // 2):
        rotL[2 * i + 1, 2 * i] = -1.0
        rotL[2 * i, 2 * i + 1] = 1.0
    rotL = rotL.astype(bf16)

    r = np.arange(P)[:, None]
    c = np.arange(P)[None, :]
    trimask = (c >= r).astype(bf16)
    ones_b = np.ones((P, P), dtype=np.float32).astype(bf16)

    wq_s = (wq * scale).astype(bf16)
    wk_s = wk.astype(bf16)
    wv_s = wv.astype(bf16)
    wo_s = wo.astype(bf16)

    xTb = [np.ascontiguousarray(x[b].T).astype(bf16) for b in range(B)]

    in_maps = []
    for cix in range(NCORES):
        b, hg = divmod(cix, NCORES // B)
        rows = slice(hg * CPC, (hg + 1) * CPC)
        blocks = []
        for h in range(HPC):
            hr = slice((hg * HPC + h) * HD, (hg * HPC + h + 1) * HD)
            blocks.append(wq_s[hr])
        for h in range(HPC):
            hr = slice((hg * HPC + h) * HD, (hg * HPC + h + 1) * HD)
            blocks.append(wk_s[hr])
        blocks.append(wv_s[rows])
        wqkvT = np.ascontiguousarray(
            np.concatenate(blocks, axis=0).T)  # [D, 1536]
        woT = np.ascontiguousarray(wo_s[:, rows].T)  # [512, D]
        in_maps.append({
            "xT": xTb[b],
            "wqkvT": wqkvT,
            "woT": woT,
            "cosT": cosT,
            "sinT": sinT,
            "rotL": rotL,
            "trimask": trimask,
            "ones": ones_b,
        })
    return in_maps


def _get_nc():
    global _nc_cache
    if _nc_cache is None:
        _nc_cache = _build_nc()
    return _nc_cache


def kernel(x, wq, wk, wv, wo, _trace=False):
    global last_exec_time_ns
    nc = _get_nc()
    in_maps = _host_inputs(np.asarray(x, dtype=np.float32),
                           np.asarray(wq, dtype=np.float32),
                           np.asarray(wk, dtype=np.float32),
                           np.asarray(wv, dtype=np.float32),
                           np.asarray(wo, dtype=np.float32))
    res = run_bass_kernel_spmd(nc, in_maps, core_ids=list(range(NCORES)),
                               trace=_trace)
    last_exec_time_ns = res.exec_time_ns
    y = np.zeros((B, S, D), dtype=np.float32)
    for cix in range(NCORES):
        b = cix // (NCORES // B)
        y[b] += res.results[cix]["yT"].T.astype(np.float32)
    return y
